# revision 9
# baseline (speedup 1.0000x reference)
"""Trainium2 Bass kernel for nn_BiLSTM2 — v2 (merged-direction rounds).

Key structural choices (driven by HW microbenchmarks: dependent DVE ops cost
~700ns each on HW regardless of width; PE/ACT chains through PSUM are cheap;
cross-engine wakes hide when instructions' deps complete before queue-head):

- tanh-only gate math: sigma(x) = (1 + tanh(x/2))/2, with the 1/2 scales folded
  into the staged weights. Gate chunk order [f, i, j, o]. State d = 2c:
      t1 = (tau_f + 1) * d          tau_* = tanh of scaled gate pre-acts
      t2 = (tau_i + 1) * tau_j
      d' = 0.5*t1 + t2
      hhat = (tau_o + 1) * tanh(d'/2)    (hhat = 2h; consumers' weights * 0.5)
  Everything uses the tanh/exp ACT table set — zero table switches, and the
  attention softmax exp can run inline per round.
- fw and bw directions share single instructions per round (one tanh ACT over
  both gate banks, one STT per tail step over both directions) — halves the
  number of waiting instructions per round vs. per-direction emission.
- online attention: when both f1[t] and b1[t] exist (round max(t, T-1-t)),
  logits + tanh + exp run immediately and es/es*h are accumulated into
  persistent PSUM banks via identity matmuls. No [H, T*B] h-slab storage, no
  O(T*B) reduces at the end. Softmax positions not covered by stored slabs are
  a host-computed constant added to the denominator.
- per-round widths M[t] = max(n_sched[t-1], n_sched[t], n_sched[T-1-t],
  n_sched[T-2-t]) (symmetric: M[t] == M[T-1-t]) so one rectangular AP covers
  both directions; indicator rows kill (i, f, o -> -30) the padded columns so
  their state pins to exact zero.
"""

import os
from contextlib import ExitStack

import numpy as np

import concourse.bass as bass
import concourse.tile as tile
import concourse.mybir as mybir
from concourse.bass import AP
from concourse.bass_utils import run_bass_kernel_spmd
from concourse.vector_clock import ScopedClock

F16 = mybir.dt.float16  # fp16: 1 cyc/row like bf16, more mantissa (values bounded)
F32 = mybir.dt.float32
AF = mybir.ActivationFunctionType
ALU = mybir.AluOpType

B, T, F, FS, H = 1024, 169, 36, 19, 128
NCORES = 8
BS = B // NCORES  # 128 rows per core
KILL = -30.0      # tanh(-30 + bounded noise) == -1 exactly in fp16

# chunk order [f, i, j, o]; reference packs [i, j, f, o]
_PERM = np.concatenate([
    np.arange(2 * H, 3 * H),  # f
    np.arange(0, H),          # i
    np.arange(H, 2 * H),      # j
    np.arange(3 * H, 4 * H),  # o
])
_GSCALE = np.concatenate([
    np.full(H, 0.5), np.full(H, 0.5), np.full(H, 1.0), np.full(H, 0.5)])
_KILLROW = np.concatenate([
    np.full(H, KILL), np.full(H, KILL), np.zeros(H), np.full(H, KILL)])


def _patch_tile_drain():
    """walrus in this container rejects >1 sem wait on the tail Drain;
    split the waits across extra NoOps (one wait each)."""

    def _drain_and_barrier(self, tick_clock, wait_clock):
        nc = self.nc
        drain_inst = nc.sync.drain()
        wait_clock.add_sem_waits(
            drain_inst.ins, ScopedClock({None: tick_clock.global_clock})
        )
        mi = drain_inst.ins
        si = mi.sync_info
        if si is not None and si.on_wait is not None and len(si.on_wait) > 1:
            waits = list(si.on_wait)
            upd = list(si.on_update) if si.on_update else []
            mi.sync_info = mybir.SyncInfo(on_wait=[waits[0]], on_update=upd)
            for w in waits[1:]:
                nop = nc.sync.nop()
                nop.ins.sync_info = mybir.SyncInfo(on_wait=[w], on_update=[])
        nc.all_engine_barrier()
        popped = nc._tile_sem_poison_stack.pop()
        assert popped is self._sem_poison
        nc.clear_and_free_semaphores(list(self.sems.allocated().values()))
        nc.all_engine_barrier()

    tile.TileContext._drain_and_barrier = _drain_and_barrier


_patch_tile_drain()


def split_multi_waits(nc, max_waits=1):
    """walrus here rejects instructions with more than ~1 sem wait; hoist
    extras onto same-engine NoOps placed immediately before the instruction."""
    import bass_rust
    nid = [0]
    for fn in nc.m.functions:
        for blk in fn.blocks:
            out = []
            changed = False
            for inst in blk.instructions:
                si = inst.sync_info
                if si is not None and si.on_wait is not None \
                        and len(si.on_wait) > max_waits:
                    waits = list(si.on_wait)
                    keep = waits[:max_waits]
                    extra = waits[max_waits:]
                    for w in extra:
                        nid[0] += 1
                        nop = bass_rust.InstNoOp(
                            name=f"waitnop-{nid[0]}", ins=[], outs=[],
                            engine=inst.engine)
                        nop.sync_info = mybir.SyncInfo(on_wait=[w],
                                                       on_update=[])
                        out.append(nop)
                    inst.sync_info = mybir.SyncInfo(
                        on_wait=keep,
                        on_update=list(si.on_update) if si.on_update else [])
                    changed = True
                out.append(inst)
            if changed:
                blk.instructions = out
    return nid[0]


def _f16(x):
    return np.ascontiguousarray(np.asarray(x).astype(np.float16))


def _f32(x):
    return np.ascontiguousarray(np.asarray(x).astype(np.float32))


def prepare(inputs):
    """Schedule + per-core input maps."""
    L = np.asarray(inputs["seq_len"]).astype(np.int64)
    order = np.argsort(-L, kind="stable")
    perms = [order[c::NCORES] for c in range(NCORES)]

    n_true = np.zeros((NCORES, T), dtype=np.int64)
    for c in range(NCORES):
        Lc = L[perms[c]]
        n_true[c] = (Lc[None, :].T > np.arange(T)[None, :]).sum(axis=0)
    n_sched = n_true.max(axis=0)
    n_sched = np.minimum(((n_sched + 3) // 4) * 4, BS).astype(np.int64)

    def nsch(t):
        if t < 0:
            return int(n_sched[0])
        if t >= T:
            return 0
        return int(n_sched[t])

    M = np.zeros(T, dtype=np.int64)
    for t in range(T):
        M[t] = max(nsch(t - 1), nsch(t), nsch(T - 1 - t), nsch(T - 2 - t))
    steps = [t for t in range(T) if M[t] > 0]
    ns = len(steps)
    assert ns > 0
    for r in range(ns):
        assert steps[ns - 1 - r] == T - 1 - steps[r], "asymmetric support"
        assert M[steps[r]] == M[steps[ns - 1 - r]]

    # xdT packing at M widths
    off_x = {}
    acc = 0
    for t in steps:
        off_x[t] = acc
        acc += int(M[t])
    CWX = max(acc, 4)

    # slab widths: with the h_prev pair-buffer scheme, slabs are only read
    # at exactly width M[t] (stage-1 x inputs, attention), so no padding
    W = {t: int(M[t]) for t in steps}
    off_w = {}
    acc = 0
    for t in steps:
        off_w[t] = acc
        acc += W[t]
    CWW = max(acc, 4)

    # stage-1 keep/scratch layout: [keepF | scratchF | scratchB | keepB]
    half = (ns - 1) / 2.0
    keepF = [steps[j] for j in range(ns) if j < half]
    keepB = [steps[j] for j in range(ns) if j > half]
    off_kf = {}
    acc = 0
    for t in keepF:
        off_kf[t] = acc
        acc += W[t]
    KF = acc
    SCF = KF            # 3 scratch F slots
    SCB = KF + 3 * BS   # 3 scratch B slots
    off_kb = {}
    acc = KF + 6 * BS
    for t in keepB:
        off_kb[t] = acc
        acc += W[t]
    H1W = max(acc, 4)
    r_att = int(np.ceil(half)) if ns > 1 else 0

    # rank-1 chunks for stage 1 (bias and/or kill rows nonzero)
    def bias_row(bb):
        b = np.asarray(bb, dtype=np.float64)[_PERM].copy()
        b[0:H] += 1.0  # forget bias, pre-halving
        return b * _GSCALE

    b1f = bias_row(inputs["bb_f1"])
    b1b = bias_row(inputs["bb_b1"])
    l1_chunks = [ch for ch in range(4)
                 if np.any(b1f[ch * H:(ch + 1) * H] != 0)
                 or np.any(b1b[ch * H:(ch + 1) * H] != 0)
                 or np.any(_KILLROW[ch * H:(ch + 1) * H] != 0)]

    def stage_w(w, hscale):
        w = np.asarray(w, dtype=np.float64)[:, _PERM] * _GSCALE[None, :] * hscale
        return _f16(w)

    def stage_wx0(wx, bb):
        w = np.asarray(wx, dtype=np.float64)[:, _PERM] * _GSCALE[None, :]
        return _f16(np.concatenate([bias_row(bb)[None, :], _KILLROW[None, :], w],
                                   axis=0))

    battv = float(np.asarray(inputs["b_att"]).reshape(-1)[0])
    cnt = np.zeros(BS, dtype=np.int64)
    for t in steps:
        cnt[0:int(M[t])] += 1
    corr = (T - cnt).astype(np.float64) * np.exp(np.tanh(battv))

    base_map = dict(
        wx0f=stage_wx0(inputs["Wx_f0"], inputs["bb_f0"]),
        wx0b=stage_wx0(inputs["Wx_b0"], inputs["bb_b0"]),
        wh0f=stage_w(inputs["Wh_f0"], 0.5),
        wh0b=stage_w(inputs["Wh_b0"], 0.5),
        wx1ft=stage_w(np.asarray(inputs["Wx_f1"])[0:H], 0.5),
        wx1fb=stage_w(np.asarray(inputs["Wx_f1"])[H:2 * H], 0.5),
        wh1f=stage_w(inputs["Wh_f1"], 0.5),
        wx1bt=stage_w(np.asarray(inputs["Wx_b1"])[0:H], 0.5),
        wx1bb=stage_w(np.asarray(inputs["Wx_b1"])[H:2 * H], 0.5),
        wh1b=stage_w(inputs["Wh_b1"], 0.5),
        bias1f=_f16(np.stack([b1f, _KILLROW])),
        bias1b=_f16(np.stack([b1b, _KILLROW])),
        wrepf=_f16(np.repeat(np.asarray(inputs["w_att"])[0:H, 0:1] * 0.5, 128,
                             axis=1)),
        wrepb=_f16(np.repeat(np.asarray(inputs["w_att"])[H:2 * H, 0:1] * 0.5, 128,
                             axis=1)),
        ident=_f16(np.eye(128)),
        battc=_f32(np.full((128, 1), battv)),
        corr=_f32(np.repeat(corr[None, :], 128, axis=0)),
        ws0=_f32(inputs["w_s0"]), ws1=_f32(inputs["w_s1"]),
        bs0=_f32(np.asarray(inputs["b_s0"]).reshape(-1, 1)),
        bs1=_f32(np.asarray(inputs["b_s1"]).reshape(-1, 1)),
        wc1s=_f32(np.asarray(inputs["w_c1"])[0:16]),
        wc1f=_f32(np.asarray(inputs["w_c1"])[16:16 + H] * 0.5),
        wc1b=_f32(np.asarray(inputs["w_c1"])[16 + H:16 + 2 * H] * 0.5),
        bc1=_f32(np.asarray(inputs["b_c1"]).reshape(-1, 1)),
        wc2=_f32(inputs["w_c2"]),
        bc2=_f32(np.asarray(inputs["b_c2"]).reshape(-1, 1)),
    )

    in_maps = []
    for c in range(NCORES):
        p = perms[c]
        Lc = L[p]
        xc = np.asarray(inputs["x_dynamic"])[p].astype(np.float32)
        tmask = (np.arange(T)[None, :] < Lc[:, None])
        xc = np.where(tmask[:, :, None], xc, 0.0)
        xcT = xc.transpose(2, 1, 0)  # [F, T, BS]
        xdT_h = np.zeros((F + 2, CWX), dtype=np.float32)
        for t in steps:
            m = int(M[t])
            o = off_x[t]
            nt = min(int(n_true[c, t]), m)
            xdT_h[2:F + 2, o:o + m] = xcT[:, t, 0:m]
            xdT_h[0, o:o + nt] = 1.0
            xdT_h[1, o + nt:o + m] = 1.0
        mm = dict(base_map)
        mm["xdT"] = _f16(xdT_h)
        mm["xsT"] = _f32(np.asarray(inputs["x_static"])[p].T)
        in_maps.append(mm)

    sched = dict(n_sched=n_sched, M=M, steps=steps, ns=ns, off_x=off_x,
                 CWX=CWX, W=W, off_w=off_w, CWW=CWW, keepF=keepF, keepB=keepB,
                 off_kf=off_kf, off_kb=off_kb, SCF=SCF, SCB=SCB, H1W=H1W,
                 r_att=r_att, l1_chunks=l1_chunks, perms=perms, n_true=n_true)
    return sched, in_maps


def _ap2(tens_ap, offset, rowpitch, m):
    return AP(tens_ap.tensor, tens_ap.offset + offset, [[rowpitch, 128], [1, m]])


def _ap3(tens_ap, offset, rowpitch, dstride, m, nd=2):
    return AP(tens_ap.tensor, tens_ap.offset + offset,
              [[rowpitch, 128], [dstride, nd], [1, m]])


def build_program(sched, reps=1, serialize=False):
    """serialize=True chains each rep's start on the previous rep's output
    (for timing runs: makes T(reps) = reps x single-shot)."""
    nc = bass.Bass("TRN2", target_bir_lowering=False, debug=False)

    steps, ns = sched["steps"], sched["ns"]
    M, off_x = sched["M"], sched["off_x"]
    W, off_w, CWW = sched["W"], sched["off_w"], sched["CWW"]
    CWX = sched["CWX"]
    off_kf, off_kb = sched["off_kf"], sched["off_kb"]
    SCF, SCB, H1W = sched["SCF"], sched["SCB"], sched["H1W"]
    r_att = sched["r_att"]
    l1_chunks = sched["l1_chunks"]
    half = (ns - 1) / 2.0

    def din(name, shape, dt):
        return nc.dram_tensor(name, shape, dt, kind="ExternalInput").ap()

    xdT = din("xdT", [F + 2, CWX], F16)
    wx0f = din("wx0f", [F + 2, 512], F16)
    wx0b = din("wx0b", [F + 2, 512], F16)
    wh0f = din("wh0f", [H, 512], F16)
    wh0b = din("wh0b", [H, 512], F16)
    wx1ft = din("wx1ft", [H, 512], F16)
    wx1fb = din("wx1fb", [H, 512], F16)
    wh1f = din("wh1f", [H, 512], F16)
    wx1bt = din("wx1bt", [H, 512], F16)
    wx1bb = din("wx1bb", [H, 512], F16)
    wh1b = din("wh1b", [H, 512], F16)
    bias1f = din("bias1f", [2, 512], F16)
    bias1b = din("bias1b", [2, 512], F16)
    wrepf = din("wrepf", [H, 128], F16)
    wrepb = din("wrepb", [H, 128], F16)
    ident = din("ident", [128, 128], F16)
    battc = din("battc", [128, 1], F32)
    corr = din("corr", [128, BS], F32)
    xsT = din("xsT", [FS, BS], F32)
    ws0 = din("ws0", [FS, 16], F32)
    ws1 = din("ws1", [16, 16], F32)
    bs0 = din("bs0", [16, 1], F32)
    bs1 = din("bs1", [16, 1], F32)
    wc1s = din("wc1s", [16, 64], F32)
    wc1f = din("wc1f", [H, 64], F32)
    wc1b = din("wc1b", [H, 64], F32)
    bc1 = din("bc1", [64, 1], F32)
    wc2 = din("wc2", [64, 32], F32)
    bc2 = din("bc2", [32, 1], F32)

    outT = nc.dram_tensor("outT", [32, BS], F32, kind="ExternalOutput").ap()

    with tile.TileContext(nc) as tc, ExitStack() as gctx:
        gpool = gctx.enter_context(tc.tile_pool(name="glob", bufs=1))
        hzero = gpool.tile([128, 2 * BS], F16, tag="hzero")
        nc.gpsimd.memset(hzero[:], 0.0)
        for _rep in range(reps):
            with ExitStack() as rctx:
                persist = rctx.enter_context(tc.tile_pool(name="persist", bufs=1))

                # ---- persistent weights / small tensors ----
                s_w = {}
                for nm, ap_, shp, dt in (
                    ("wx0f", wx0f, [F + 2, 512], F16),
                    ("wx0b", wx0b, [F + 2, 512], F16),
                    ("wh0f", wh0f, [H, 512], F16),
                    ("wh0b", wh0b, [H, 512], F16),
                    ("xdT", xdT, [F + 2, CWX], F16),
                    ("wx1ft", wx1ft, [H, 512], F16),
                    ("wx1fb", wx1fb, [H, 512], F16),
                    ("wh1f", wh1f, [H, 512], F16),
                    ("wx1bt", wx1bt, [H, 512], F16),
                    ("wx1bb", wx1bb, [H, 512], F16),
                    ("wh1b", wh1b, [H, 512], F16),
                    ("bias1f", bias1f, [2, 512], F16),
                    ("bias1b", bias1b, [2, 512], F16),
                    ("wrepf", wrepf, [H, 128], F16),
                    ("wrepb", wrepb, [H, 128], F16),
                    ("ident", ident, [128, 128], F16),
                    ("battc", battc, [128, 1], F32),
                    ("corr", corr, [128, BS], F32),
                    ("xsT", xsT, [FS, BS], F32),
                    ("ws0", ws0, [FS, 16], F32),
                    ("ws1", ws1, [16, 16], F32),
                    ("bs0", bs0, [16, 1], F32),
                    ("bs1", bs1, [16, 1], F32),
                    ("wc1s", wc1s, [16, 64], F32),
                    ("wc1f", wc1f, [H, 64], F32),
                    ("wc1b", wc1b, [H, 64], F32),
                    ("bc1", bc1, [64, 1], F32),
                    ("wc2", wc2, [64, 32], F32),
                    ("bc2", bc2, [32, 1], F32),
                ):
                    if nm == "xdT":
                        s_w[nm] = persist.tile(shp, dt, tag="t_" + nm,
                                               name="t_" + nm)
                        # alternating head/tail chunks: fw needs the head
                        # first, bw the tail
                        NCH = 16
                        bounds = [round(i * CWX / NCH) for i in range(NCH + 1)]
                        lo, hi = 0, NCH - 1
                        ordr = []
                        while lo <= hi:
                            ordr.append(lo)
                            if hi != lo:
                                ordr.append(hi)
                            lo += 1
                            hi -= 1
                        for ci in ordr:
                            a, b_ = bounds[ci], bounds[ci + 1]
                            if b_ > a:
                                nc.sync.dma_start(out=s_w[nm][:, a:b_],
                                                  in_=ap_[:, a:b_])
                    else:
                        s_w[nm] = persist.tile(shp, dt, tag="t_" + nm,
                                               name="t_" + nm)
                        nc.sync.dma_start(out=s_w[nm][:], in_=ap_[:])

                fw0T = persist.tile([128, CWW], F16, tag="fw0T")
                bw0T = persist.tile([128, CWW], F16, tag="bw0T")
                sT = persist.tile([16, BS], F32, tag="sT")

                fw0_pitch = fw0T[:].ap[0][0]
                bw0_pitch = bw0T[:].ap[0][0]

                # ---- static branch ----
                with tc.tile_pool(name="ps_static", bufs=1, space="PSUM") as pss:
                    ps1 = pss.tile([16, BS], F32, tag="pst1")
                    nc.tensor.matmul(ps1[:], s_w["ws0"][:], s_w["xsT"][:],
                                     start=True, stop=True)
                    s0 = persist.tile([16, BS], F32, tag="s0tmp")
                    nc.scalar.activation(s0[:], ps1[:], AF.Relu, bias=s_w["bs0"][:])
                    ps2 = pss.tile([16, BS], F32, tag="pst2")
                    nc.tensor.matmul(ps2[:], s_w["ws1"][:], s0[:], start=True,
                                     stop=True)
                    nc.scalar.activation(sT[:], ps2[:], AF.Relu, bias=s_w["bs1"][:])

                def emit_stage(stage, sctx, h1all=None, att_state=None):
                    """Emit all rounds of one stage (0 or 1)."""
                    pgate = sctx.enter_context(
                        tc.tile_pool(name=f"psg{stage}", bufs=1, space="PSUM"))
                    pq = sctx.enter_context(
                        tc.tile_pool(name=f"pq{stage}", bufs=1, space="PSUM"))
                    tfi = pq.tile([128, 512], F32, tag=f"tfi{stage}",
                                  name=f"tfi{stage}")
                    t1th = pq.tile([128, 512], F32, tag=f"t1th{stage}",
                                   name=f"t1th{stage}")
                    psig = sctx.enter_context(
                        tc.tile_pool(name=f"sig{stage}", bufs=3))
                    ptmp = sctx.enter_context(
                        tc.tile_pool(name=f"tmp{stage}", bufs=4))
                    pd = sctx.enter_context(tc.tile_pool(name=f"d{stage}", bufs=1))
                    dp = [pd.tile([128, 2 * BS], F16, tag=f"d{stage}_{k}",
                                  name=f"d{stage}_{k}") for k in range(2)]
                    hpair = [pd.tile([128, 2 * BS], F16, tag=f"hp{stage}_{k}",
                                     name=f"hp{stage}_{k}") for k in range(2)]
                    for k in range(2):
                        nc.gpsimd.memset(dp[k][:], 0.0)
                        nc.gpsimd.memset(hpair[k][:], 0.0)

                    if stage == 1:
                        patt = sctx.enter_context(
                            tc.tile_pool(name="attp", bufs=3))
                        psl_pool = sctx.enter_context(
                            tc.tile_pool(name="psl", bufs=1, space="PSUM"))
                        h1_pitch = h1all[:].ap[0][0]

                    def f_addr(j):
                        # fw slab idx j within stage-1 store
                        t = steps[j]
                        if j < half:
                            return off_kf[t]
                        return SCF + (j % 3) * BS

                    def b_addr(j):
                        t = steps[j]
                        if j > half:
                            return off_kb[t]
                        return SCB + ((ns - 1 - j) % 3) * BS

                    def emit_attention(r, last=False):
                        """Attention for the slab pair completed at round r
                        (emitted one round later to keep PE free-running)."""
                        jhi, jlo = r, ns - 1 - r
                        m = int(M[steps[jhi]])
                        single = (jhi == jlo)
                        nsl = 1 if single else 2
                        hfr = hpair[r % 2]  # round r's fresh outputs
                        psl = psl_pool.tile([128, 512], F32, tag="psl")
                        # slab order [lo, hi] so both manual-AP pairs ascend;
                        # the round's fresh sides read the pair buffer (no
                        # wait on the Pool slab copies)
                        pairs = ([(jlo, 0), (jhi, 1)] if not single
                                 else [(jhi, 0)])
                        for j, sl in pairs:
                            f_src = (hfr[:, 0:m] if j == jhi
                                     else _ap2(h1all[:], f_addr(j), h1_pitch, m))
                            b_src = (hfr[:, BS:BS + m] if j == jlo or single
                                     else _ap2(h1all[:], b_addr(j), h1_pitch, m))
                            nc.tensor.matmul(
                                psl[:, sl * 128:sl * 128 + m], s_w["wrepf"][:],
                                f_src, start=True, stop=False)
                            nc.tensor.matmul(
                                psl[:, sl * 128:sl * 128 + m], s_w["wrepb"][:],
                                b_src, start=False, stop=True)
                        thl = patt.tile([128, 256], F16, tag="thl")
                        esl = patt.tile([128, 256], F16, tag="esl")
                        pin = psl[:].rearrange("p (s n) -> p s n", s=4)[:, 0:nsl, 0:m]
                        tout = thl[:].rearrange("p (s n) -> p s n", s=2)[:, 0:nsl, 0:m]
                        nc.scalar.activation(tout, pin, AF.Tanh, bias=s_w["battc"][:])
                        eout = esl[:].rearrange("p (s n) -> p s n", s=2)[:, 0:nsl, 0:m]
                        nc.scalar.activation(eout, tout, AF.Exp)
                        # ws = es * slab (both dirs)
                        wsf = patt.tile([128, 256], F16, tag="wsf")
                        wsb = patt.tile([128, 256], F16, tag="wsb")
                        if single:
                            f_ap = _ap2(h1all[:], f_addr(jhi), h1_pitch, m)
                            b_ap = _ap2(h1all[:], b_addr(jhi), h1_pitch, m)
                        else:
                            f0, f1_ = f_addr(jlo), f_addr(jhi)
                            b0, b1_ = b_addr(jlo), b_addr(jhi)
                            assert f1_ > f0 and b1_ > b0
                            f_ap = _ap3(h1all[:], f0, h1_pitch, f1_ - f0, m)
                            b_ap = _ap3(h1all[:], b0, h1_pitch, b1_ - b0, m)
                        wf = wsf[:].rearrange("p (s n) -> p s n", s=2)[:, 0:nsl, 0:m]
                        wb = wsb[:].rearrange("p (s n) -> p s n", s=2)[:, 0:nsl, 0:m]
                        ein = esl[:].rearrange("p (s n) -> p s n", s=2)[:, 0:nsl, 0:m]
                        if single:
                            f_ap = _ap3(h1all[:], f_addr(jhi), h1_pitch, 1, m,
                                        nd=1)
                            b_ap = _ap3(h1all[:], b_addr(jhi), h1_pitch, 1, m,
                                        nd=1)
                        # ws mults on Pool: off the DVE critical path
                        nc.gpsimd.tensor_tensor(wf, f_ap, ein, ALU.mult)
                        nc.gpsimd.tensor_tensor(wb, b_ap, ein, ALU.mult)
                        # accumulate into persistent PSUM banks via identity
                        # MMs; regions must be uniformly fresh-or-accumulating,
                        # so split on the per-bank high-water mark when the
                        # round width grows.
                        for sl in range(nsl):
                            for kind, src in (("d", esl), ("f", wsf),
                                              ("b", wsb)):
                                acc = att_state[kind]
                                mw = att_state["mw"][kind]
                                stop = (last and sl == nsl - 1)
                                if att_state["first"][kind]:
                                    nc.tensor.matmul(
                                        acc[:, 0:m], s_w["ident"][:],
                                        src[:, sl * 128:sl * 128 + m],
                                        start=True, stop=stop)
                                    att_state["first"][kind] = False
                                elif m > mw:
                                    nc.tensor.matmul(
                                        acc[:, 0:mw], s_w["ident"][:],
                                        src[:, sl * 128:sl * 128 + mw],
                                        start=False, stop=False)
                                    nc.tensor.matmul(
                                        acc[:, mw:m], s_w["ident"][:],
                                        src[:, sl * 128 + mw:sl * 128 + m],
                                        start=False, stop=stop)
                                else:
                                    nc.tensor.matmul(
                                        acc[:, 0:m], s_w["ident"][:],
                                        src[:, sl * 128:sl * 128 + m],
                                        start=False, stop=stop)
                                att_state["mw"][kind] = max(mw, m)

                    for r in range(ns):
                        tf = steps[r]
                        tb = steps[ns - 1 - r]
                        m = int(M[tf])
                        ps = pgate.tile([128, 1024], F32, tag=f"ps{stage}",
                                        name=f"ps{stage}")
                        # ---- gate matmuls: x-parts first, Wh last ----
                        if stage == 0:
                            for di, (wx, xo) in enumerate(
                                    ((s_w["wx0f"], off_x[tf]),
                                     (s_w["wx0b"], off_x[tb]))):
                                for c in range(4):
                                    nc.tensor.matmul(
                                        ps[:, di * 512 + c * 128:
                                           di * 512 + c * 128 + m],
                                        wx[:, c * 128:(c + 1) * 128],
                                        s_w["xdT"][:, xo:xo + m],
                                        start=(c == 0), stop=False)
                        else:
                            for di, (wt, wb_, bt, tx) in enumerate((
                                    (s_w["wx1ft"], s_w["wx1fb"], s_w["bias1f"], tf),
                                    (s_w["wx1bt"], s_w["wx1bb"], s_w["bias1b"], tb))):
                                fsl = _ap2(fw0T[:], off_w[tx], fw0_pitch, m)
                                bsl = _ap2(bw0T[:], off_w[tx], bw0_pitch, m)
                                for c in range(4):
                                    nc.tensor.matmul(
                                        ps[:, di * 512 + c * 128:
                                           di * 512 + c * 128 + m],
                                        wt[:, c * 128:(c + 1) * 128], fsl,
                                        start=(c == 0), stop=False)
                                for c in range(4):
                                    nc.tensor.matmul(
                                        ps[:, di * 512 + c * 128:
                                           di * 512 + c * 128 + m],
                                        wb_[:, c * 128:(c + 1) * 128], bsl,
                                        start=False, stop=False)
                                for c in l1_chunks:
                                    nc.tensor.matmul(
                                        ps[:, di * 512 + c * 128:
                                           di * 512 + c * 128 + m],
                                        bt[:, c * 128:(c + 1) * 128],
                                        s_w["xdT"][0:2, off_x[tx]:off_x[tx] + m],
                                        start=False, stop=False)
                        # attention for the previous round (PE keeps running)
                        if stage == 1 and r - 1 >= r_att:
                            emit_attention(r - 1)
                        # Wh (recurrent, on the critical path); h_prev comes
                        # from the previous round's pair buffer. o-gate chunk
                        # (3) emitted last so tau_fij only waits on chunks 0-2.
                        hpv = hzero if r == 0 else hpair[(r - 1) % 2]
                        whf = s_w["wh0f"] if stage == 0 else s_w["wh1f"]
                        whb = s_w["wh0b"] if stage == 0 else s_w["wh1b"]
                        dirs = ((0, whf, hpv[:, 0:m]),
                                (1, whb, hpv[:, BS:BS + m]))
                        for c in (0, 1, 2, 3):
                            for di, wh, hprev in dirs:
                                nc.tensor.matmul(
                                    ps[:, di * 512 + c * 128:
                                       di * 512 + c * 128 + m],
                                    wh[:, c * 128:(c + 1) * 128], hprev,
                                    start=False,
                                    stop=(c == 3))
                        # ---- tau, PSUM-routed: f,i chunks -> PSUM (feeds the
                        # DVE tail without the SBUF read-after-write bubble),
                        # j,o chunks -> SBUF ----
                        sig = psig.tile([128, 512], F16, tag=f"sig{stage}",
                                        name=f"sig{stage}")
                        gp = ps[:].rearrange("p (d c n) -> p d c n", d=2, c=4)
                        tfir = tfi[:].rearrange("p (d c n) -> p d c n", d=2, c=2)
                        sjor = sig[:].rearrange("p (d c n) -> p d c n", d=2, c=2)
                        nc.scalar.activation(tfir[:, :, :, 0:m],
                                             gp[:, :, 0:2, 0:m], AF.Tanh)
                        nc.scalar.activation(sjor[:, :, :, 0:m],
                                             gp[:, :, 2:4, 0:m], AF.Tanh)
                        # ---- tail (merged dirs): t1, theta in PSUM ----
                        dprev, dnew = dp[r % 2], dp[(r + 1) % 2]
                        t2 = ptmp.tile([128, 256], F16, tag="t2")
                        dpr = dprev[:].rearrange("p (d n) -> p d n", d=2)
                        dnw = dnew[:].rearrange("p (d n) -> p d n", d=2)
                        t1r = t1th[:, 0:256].rearrange("p (d n) -> p d n", d=2)
                        thr = t1th[:, 256:512].rearrange("p (d n) -> p d n", d=2)
                        t2r = t2[:].rearrange("p (d n) -> p d n", d=2)
                        nc.vector.scalar_tensor_tensor(
                            t1r[:, :, 0:m], tfir[:, :, 0, 0:m], 1.0,
                            dpr[:, :, 0:m], ALU.add, ALU.mult)
                        nc.vector.scalar_tensor_tensor(
                            t2r[:, :, 0:m], tfir[:, :, 1, 0:m], 1.0,
                            sjor[:, :, 0, 0:m], ALU.add, ALU.mult)
                        nc.vector.scalar_tensor_tensor(
                            dnw[:, :, 0:m], t1r[:, :, 0:m], 0.5,
                            t2r[:, :, 0:m], ALU.mult, ALU.add)
                        nc.scalar.activation(thr[:, :, 0:m], dnw[:, :, 0:m],
                                             AF.Tanh, scale=0.5)
                        # hhat = (tau_o + 1) * th -> pair buffer (feeds next
                        # round's Wh MMs directly)
                        hcur = hpair[r % 2]
                        hcr = hcur[:].rearrange("p (d n) -> p d n", d=2)
                        nc.vector.scalar_tensor_tensor(
                            hcr[:, :, 0:m], sjor[:, :, 1, 0:m], 1.0,
                            thr[:, :, 0:m], ALU.add, ALU.mult)
                        # off-chain slab fills on Pool (consumed rounds later)
                        if stage == 0:
                            fdst = _ap2(fw0T[:], off_w[tf], fw0_pitch, m)
                            bdst = _ap2(bw0T[:], off_w[tb], bw0_pitch, m)
                        else:
                            fdst = _ap2(h1all[:], f_addr(r), h1_pitch, m)
                            bdst = _ap2(h1all[:], b_addr(ns - 1 - r),
                                        h1_pitch, m)
                        nc.gpsimd.tensor_copy(fdst, hcur[:, 0:m])
                        nc.gpsimd.tensor_copy(bdst, hcur[:, BS:BS + m])
                    # trailing attention rounds
                    if stage == 1:
                        for r in range(max(r_att, ns - 1), ns):
                            emit_attention(r, last=(r == ns - 1))

                # ================= stage 0 =================
                with ExitStack() as sctx:
                    emit_stage(0, sctx)

                # ================= stage 1 + attention =================
                att_f = persist.tile([H, BS], F32, tag="att_f")
                att_b = persist.tile([H, BS], F32, tag="att_b")
                den_s = persist.tile([128, BS], F32, tag="den_s")
                with ExitStack() as sctx:
                    h1all = sctx.enter_context(
                        tc.tile_pool(name="h1pool", bufs=1)
                    ).tile([128, H1W], F16, tag="h1all")
                    pacc = sctx.enter_context(
                        tc.tile_pool(name="acc", bufs=1, space="PSUM"))
                    accd = pacc.tile([128, 512], F32, tag="accd")
                    accf = pacc.tile([128, 512], F32, tag="accf")
                    accb = pacc.tile([128, 512], F32, tag="accb")
                    att_state = {"d": accd, "f": accf, "b": accb,
                                 "first": {"d": True, "f": True, "b": True},
                                 "mw": {"d": 0, "f": 0, "b": 0}}
                    emit_stage(1, sctx, h1all=h1all, att_state=att_state)
                    # evacuate attention accumulators
                    nc.vector.tensor_tensor(den_s[:], accd[:, 0:BS],
                                            s_w["corr"][:], ALU.add)
                    rd = persist.tile([128, BS], F32, tag="rd")
                    nc.vector.reciprocal(rd[:], den_s[:])
                    nc.vector.tensor_tensor(att_f[:], accf[:, 0:BS],
                                            rd[:], ALU.mult)
                    nc.vector.tensor_tensor(att_b[:], accb[:, 0:BS],
                                            rd[:], ALU.mult)

                # ================= classifier =================
                with ExitStack() as cctx:
                    pcl = cctx.enter_context(tc.tile_pool(name="cls", bufs=1))
                    psc = cctx.enter_context(
                        tc.tile_pool(name="psum_cls", bufs=1, space="PSUM"))
                    ph = psc.tile([64, BS], F32, tag="ph")
                    nc.tensor.matmul(ph[:], s_w["wc1s"][:], sT[:], start=True,
                                     stop=False)
                    nc.tensor.matmul(ph[:], s_w["wc1f"][:], att_f[:], start=False,
                                     stop=False)
                    nc.tensor.matmul(ph[:], s_w["wc1b"][:], att_b[:], start=False,
                                     stop=True)
                    h1t = pcl.tile([64, BS], F32, tag="h1t")
                    nc.scalar.activation(h1t[:], ph[:], AF.Relu, bias=s_w["bc1"][:])
                    po = psc.tile([32, BS], F32, tag="po")
                    nc.tensor.matmul(po[:], s_w["wc2"][:], h1t[:], start=True,
                                     stop=True)
                    oT = pcl.tile([32, BS], F32, tag="oT")
                    nc.scalar.activation(oT[:], po[:], AF.Relu, bias=s_w["bc2"][:])
                    nc.sync.dma_start(out=outT[:], in_=oT[:])
                    if serialize and _rep < reps - 1:
                        # write zeros that depend on this rep's output into
                        # hzero, so the next rep's round 0 waits for it
                        nc.vector.scalar_tensor_tensor(
                            hzero[0:32, 0:BS], oT[:], 0.0, oT[:],
                            ALU.mult, ALU.mult)

    return nc


def kernel(x_static, x_dynamic, seq_len, w_s0, b_s0, w_s1, b_s1,
           Wx_f0, Wh_f0, bb_f0, Wx_b0, Wh_b0, bb_b0,
           Wx_f1, Wh_f1, bb_f1, Wx_b1, Wh_b1, bb_b1,
           w_att, b_att, w_c1, b_c1, w_c2, b_c2):
    inputs = dict(
        x_static=x_static, x_dynamic=x_dynamic, seq_len=seq_len,
        w_s0=w_s0, b_s0=b_s0, w_s1=w_s1, b_s1=b_s1,
        Wx_f0=Wx_f0, Wh_f0=Wh_f0, bb_f0=bb_f0,
        Wx_b0=Wx_b0, Wh_b0=Wh_b0, bb_b0=bb_b0,
        Wx_f1=Wx_f1, Wh_f1=Wh_f1, bb_f1=bb_f1,
        Wx_b1=Wx_b1, Wh_b1=Wh_b1, bb_b1=bb_b1,
        w_att=w_att, b_att=b_att, w_c1=w_c1, b_c1=b_c1,
        w_c2=w_c2, b_c2=b_c2,
    )
    sched, in_maps = prepare(inputs)
    nc = build_program(sched)
    split_multi_waits(nc, max_waits=1)

    trace = os.environ.get("TRN_KERNEL_TRACE", "0") == "1"
    try:
        res = run_bass_kernel_spmd(nc, in_maps, list(range(NCORES)), trace=trace)
    except ModuleNotFoundError:
        res = run_bass_kernel_spmd(nc, in_maps, list(range(NCORES)))
    if trace:
        kernel.last_results = res

    out = np.zeros((B, 32), dtype=np.float32)
    for c in range(NCORES):
        out[sched["perms"][c]] = res.results[c]["outT"].T
    return out


# revision 14
# speedup vs baseline: 1.0113x; 1.0113x over previous
"""Trainium2 Bass kernel for nn_BiLSTM2 — v2 (merged-direction rounds).

Key structural choices (driven by HW microbenchmarks: dependent DVE ops cost
~700ns each on HW regardless of width; PE/ACT chains through PSUM are cheap;
cross-engine wakes hide when instructions' deps complete before queue-head):

- tanh-only gate math: sigma(x) = (1 + tanh(x/2))/2, with the 1/2 scales folded
  into the staged weights. Gate chunk order [f, i, j, o]. State d = 2c:
      t1 = (tau_f + 1) * d          tau_* = tanh of scaled gate pre-acts
      t2 = (tau_i + 1) * tau_j
      d' = 0.5*t1 + t2
      hhat = (tau_o + 1) * tanh(d'/2)    (hhat = 2h; consumers' weights * 0.5)
  Everything uses the tanh/exp ACT table set — zero table switches, and the
  attention softmax exp can run inline per round.
- fw and bw directions share single instructions per round (one tanh ACT over
  both gate banks, one STT per tail step over both directions) — halves the
  number of waiting instructions per round vs. per-direction emission.
- online attention: when both f1[t] and b1[t] exist (round max(t, T-1-t)),
  logits + tanh + exp run immediately and es/es*h are accumulated into
  persistent PSUM banks via identity matmuls. No [H, T*B] h-slab storage, no
  O(T*B) reduces at the end. Softmax positions not covered by stored slabs are
  a host-computed constant added to the denominator.
- per-round widths M[t] = max(n_sched[t-1], n_sched[t], n_sched[T-1-t],
  n_sched[T-2-t]) (symmetric: M[t] == M[T-1-t]) so one rectangular AP covers
  both directions; indicator rows kill (i, f, o -> -30) the padded columns so
  their state pins to exact zero.
"""

import os
from contextlib import ExitStack

import numpy as np

import concourse.bass as bass
import concourse.tile as tile
import concourse.mybir as mybir
from concourse.bass import AP
from concourse.bass_utils import run_bass_kernel_spmd
from concourse.vector_clock import ScopedClock

F16 = mybir.dt.float16  # fp16: 1 cyc/row like bf16, more mantissa (values bounded)
F32 = mybir.dt.float32
AF = mybir.ActivationFunctionType
ALU = mybir.AluOpType

B, T, F, FS, H = 1024, 169, 36, 19, 128
NCORES = 8
BS = B // NCORES  # 128 rows per core
KILL = -30.0      # tanh(-30 + bounded noise) == -1 exactly in fp16

# chunk order [f, i, j, o]; reference packs [i, j, f, o]
_PERM = np.concatenate([
    np.arange(2 * H, 3 * H),  # f
    np.arange(0, H),          # i
    np.arange(H, 2 * H),      # j
    np.arange(3 * H, 4 * H),  # o
])
_GSCALE = np.concatenate([
    np.full(H, 0.5), np.full(H, 0.5), np.full(H, 1.0), np.full(H, 0.5)])
_KILLROW = np.concatenate([
    np.full(H, KILL), np.full(H, KILL), np.zeros(H), np.full(H, KILL)])


def _patch_tile_drain():
    """walrus in this container rejects >1 sem wait on the tail Drain;
    split the waits across extra NoOps (one wait each)."""

    def _drain_and_barrier(self, tick_clock, wait_clock):
        nc = self.nc
        drain_inst = nc.sync.drain()
        wait_clock.add_sem_waits(
            drain_inst.ins, ScopedClock({None: tick_clock.global_clock})
        )
        mi = drain_inst.ins
        si = mi.sync_info
        if si is not None and si.on_wait is not None and len(si.on_wait) > 1:
            waits = list(si.on_wait)
            upd = list(si.on_update) if si.on_update else []
            mi.sync_info = mybir.SyncInfo(on_wait=[waits[0]], on_update=upd)
            for w in waits[1:]:
                nop = nc.sync.nop()
                nop.ins.sync_info = mybir.SyncInfo(on_wait=[w], on_update=[])
        nc.all_engine_barrier()
        popped = nc._tile_sem_poison_stack.pop()
        assert popped is self._sem_poison
        nc.clear_and_free_semaphores(list(self.sems.allocated().values()))
        nc.all_engine_barrier()

    tile.TileContext._drain_and_barrier = _drain_and_barrier


_patch_tile_drain()


def split_multi_waits(nc, max_waits=1):
    """walrus here rejects instructions with more than ~1 sem wait; hoist
    extras onto same-engine NoOps placed immediately before the instruction."""
    import bass_rust
    nid = [0]
    for fn in nc.m.functions:
        for blk in fn.blocks:
            out = []
            changed = False
            for inst in blk.instructions:
                si = inst.sync_info
                if si is not None and si.on_wait is not None \
                        and len(si.on_wait) > max_waits:
                    waits = list(si.on_wait)
                    keep = waits[:max_waits]
                    extra = waits[max_waits:]
                    for w in extra:
                        nid[0] += 1
                        nop = bass_rust.InstNoOp(
                            name=f"waitnop-{nid[0]}", ins=[], outs=[],
                            engine=inst.engine)
                        nop.sync_info = mybir.SyncInfo(on_wait=[w],
                                                       on_update=[])
                        out.append(nop)
                    inst.sync_info = mybir.SyncInfo(
                        on_wait=keep,
                        on_update=list(si.on_update) if si.on_update else [])
                    changed = True
                out.append(inst)
            if changed:
                blk.instructions = out
    return nid[0]


def _f16(x):
    return np.ascontiguousarray(np.asarray(x).astype(np.float16))


def _f32(x):
    return np.ascontiguousarray(np.asarray(x).astype(np.float32))


def prepare(inputs):
    """Schedule + per-core input maps."""
    L = np.asarray(inputs["seq_len"]).astype(np.int64)
    order = np.argsort(-L, kind="stable")
    perms = [order[c::NCORES] for c in range(NCORES)]

    n_true = np.zeros((NCORES, T), dtype=np.int64)
    for c in range(NCORES):
        Lc = L[perms[c]]
        n_true[c] = (Lc[None, :].T > np.arange(T)[None, :]).sum(axis=0)
    n_sched = n_true.max(axis=0)
    n_sched = np.minimum(((n_sched + 3) // 4) * 4, BS).astype(np.int64)

    def nsch(t):
        if t < 0:
            return int(n_sched[0])
        if t >= T:
            return 0
        return int(n_sched[t])

    M = np.zeros(T, dtype=np.int64)
    for t in range(T):
        M[t] = max(nsch(t - 1), nsch(t), nsch(T - 1 - t), nsch(T - 2 - t))
    steps = [t for t in range(T) if M[t] > 0]
    ns = len(steps)
    assert ns > 0
    for r in range(ns):
        assert steps[ns - 1 - r] == T - 1 - steps[r], "asymmetric support"
        assert M[steps[r]] == M[steps[ns - 1 - r]]

    # xdT packing at M widths
    off_x = {}
    acc = 0
    for t in steps:
        off_x[t] = acc
        acc += int(M[t])
    CWX = max(acc, 4)

    # slab widths: with the h_prev pair-buffer scheme, slabs are only read
    # at exactly width M[t] (stage-1 x inputs, attention), so no padding
    W = {t: int(M[t]) for t in steps}
    off_w = {}
    acc = 0
    for t in steps:
        off_w[t] = acc
        acc += W[t]
    CWW = max(acc, 4)

    # stage-1 keep/scratch layout: [keepF | scratchF | scratchB | keepB]
    half = (ns - 1) / 2.0
    keepF = [steps[j] for j in range(ns) if j < half]
    keepB = [steps[j] for j in range(ns) if j > half]
    off_kf = {}
    acc = 0
    for t in keepF:
        off_kf[t] = acc
        acc += W[t]
    KF = acc
    SCF = KF            # 3 scratch F slots
    SCB = KF + 3 * BS   # 3 scratch B slots
    off_kb = {}
    acc = KF + 6 * BS
    for t in keepB:
        off_kb[t] = acc
        acc += W[t]
    H1W = max(acc, 4)
    r_att = int(np.ceil(half)) if ns > 1 else 0

    # rank-1 chunks for stage 1 (bias and/or kill rows nonzero)
    def bias_row(bb):
        b = np.asarray(bb, dtype=np.float64)[_PERM].copy()
        b[0:H] += 1.0  # forget bias, pre-halving
        return b * _GSCALE

    b1f = bias_row(inputs["bb_f1"])
    b1b = bias_row(inputs["bb_b1"])
    l1_chunks = [ch for ch in range(4)
                 if np.any(b1f[ch * H:(ch + 1) * H] != 0)
                 or np.any(b1b[ch * H:(ch + 1) * H] != 0)
                 or np.any(_KILLROW[ch * H:(ch + 1) * H] != 0)]

    def stage_w(w, hscale):
        w = np.asarray(w, dtype=np.float64)[:, _PERM] * _GSCALE[None, :] * hscale
        return _f16(w)

    def stage_wx0(wx, bb):
        w = np.asarray(wx, dtype=np.float64)[:, _PERM] * _GSCALE[None, :]
        return _f16(np.concatenate([bias_row(bb)[None, :], _KILLROW[None, :], w],
                                   axis=0))

    battv = float(np.asarray(inputs["b_att"]).reshape(-1)[0])
    cnt = np.zeros(BS, dtype=np.int64)
    for t in steps:
        cnt[0:int(M[t])] += 1
    corr = (T - cnt).astype(np.float64) * np.exp(np.tanh(battv))

    base_map = dict(
        wx0f=stage_wx0(inputs["Wx_f0"], inputs["bb_f0"]),
        wx0b=stage_wx0(inputs["Wx_b0"], inputs["bb_b0"]),
        wh0f=stage_w(inputs["Wh_f0"], 0.5),
        wh0b=stage_w(inputs["Wh_b0"], 0.5),
        wx1ft=stage_w(np.asarray(inputs["Wx_f1"])[0:H], 0.5),
        wx1fb=stage_w(np.asarray(inputs["Wx_f1"])[H:2 * H], 0.5),
        wh1f=stage_w(inputs["Wh_f1"], 0.5),
        wx1bt=stage_w(np.asarray(inputs["Wx_b1"])[0:H], 0.5),
        wx1bb=stage_w(np.asarray(inputs["Wx_b1"])[H:2 * H], 0.5),
        wh1b=stage_w(inputs["Wh_b1"], 0.5),
        bias1f=_f16(np.stack([b1f, _KILLROW])),
        bias1b=_f16(np.stack([b1b, _KILLROW])),
        wrepf=_f16(np.repeat(np.asarray(inputs["w_att"])[0:H, 0:1] * 0.5, 128,
                             axis=1)),
        wrepb=_f16(np.repeat(np.asarray(inputs["w_att"])[H:2 * H, 0:1] * 0.5, 128,
                             axis=1)),
        ident=_f16(np.eye(128)),
        battc=_f32(np.full((128, 1), battv)),
        corr=_f32(np.repeat(corr[None, :], 128, axis=0)),
        ws0=_f32(inputs["w_s0"]), ws1=_f32(inputs["w_s1"]),
        bs0=_f32(np.asarray(inputs["b_s0"]).reshape(-1, 1)),
        bs1=_f32(np.asarray(inputs["b_s1"]).reshape(-1, 1)),
        wc1s=_f32(np.asarray(inputs["w_c1"])[0:16]),
        wc1f=_f32(np.asarray(inputs["w_c1"])[16:16 + H] * 0.5),
        wc1b=_f32(np.asarray(inputs["w_c1"])[16 + H:16 + 2 * H] * 0.5),
        bc1=_f32(np.asarray(inputs["b_c1"]).reshape(-1, 1)),
        wc2=_f32(inputs["w_c2"]),
        bc2=_f32(np.asarray(inputs["b_c2"]).reshape(-1, 1)),
    )

    in_maps = []
    for c in range(NCORES):
        p = perms[c]
        Lc = L[p]
        xc = np.asarray(inputs["x_dynamic"])[p].astype(np.float32)
        tmask = (np.arange(T)[None, :] < Lc[:, None])
        xc = np.where(tmask[:, :, None], xc, 0.0)
        xcT = xc.transpose(2, 1, 0)  # [F, T, BS]
        xdT_h = np.zeros((F + 2, CWX), dtype=np.float32)
        for t in steps:
            m = int(M[t])
            o = off_x[t]
            nt = min(int(n_true[c, t]), m)
            xdT_h[2:F + 2, o:o + m] = xcT[:, t, 0:m]
            xdT_h[0, o:o + nt] = 1.0
            xdT_h[1, o + nt:o + m] = 1.0
        mm = dict(base_map)
        mm["xdT"] = _f16(xdT_h)
        mm["xsT"] = _f32(np.asarray(inputs["x_static"])[p].T)
        in_maps.append(mm)

    sched = dict(n_sched=n_sched, M=M, steps=steps, ns=ns, off_x=off_x,
                 CWX=CWX, W=W, off_w=off_w, CWW=CWW, keepF=keepF, keepB=keepB,
                 off_kf=off_kf, off_kb=off_kb, SCF=SCF, SCB=SCB, H1W=H1W,
                 r_att=r_att, l1_chunks=l1_chunks, perms=perms, n_true=n_true)
    return sched, in_maps


def _ap2(tens_ap, offset, rowpitch, m):
    return AP(tens_ap.tensor, tens_ap.offset + offset, [[rowpitch, 128], [1, m]])


def _ap3(tens_ap, offset, rowpitch, dstride, m, nd=2):
    return AP(tens_ap.tensor, tens_ap.offset + offset,
              [[rowpitch, 128], [dstride, nd], [1, m]])


def build_program(sched, reps=1, serialize=False):
    """serialize=True chains each rep's start on the previous rep's output
    (for timing runs: makes T(reps) = reps x single-shot)."""
    nc = bass.Bass("TRN2", target_bir_lowering=False, debug=False)

    steps, ns = sched["steps"], sched["ns"]
    M, off_x = sched["M"], sched["off_x"]
    W, off_w, CWW = sched["W"], sched["off_w"], sched["CWW"]
    CWX = sched["CWX"]
    off_kf, off_kb = sched["off_kf"], sched["off_kb"]
    SCF, SCB, H1W = sched["SCF"], sched["SCB"], sched["H1W"]
    r_att = sched["r_att"]
    l1_chunks = sched["l1_chunks"]
    half = (ns - 1) / 2.0

    def din(name, shape, dt):
        return nc.dram_tensor(name, shape, dt, kind="ExternalInput").ap()

    xdT = din("xdT", [F + 2, CWX], F16)
    wx0f = din("wx0f", [F + 2, 512], F16)
    wx0b = din("wx0b", [F + 2, 512], F16)
    wh0f = din("wh0f", [H, 512], F16)
    wh0b = din("wh0b", [H, 512], F16)
    wx1ft = din("wx1ft", [H, 512], F16)
    wx1fb = din("wx1fb", [H, 512], F16)
    wh1f = din("wh1f", [H, 512], F16)
    wx1bt = din("wx1bt", [H, 512], F16)
    wx1bb = din("wx1bb", [H, 512], F16)
    wh1b = din("wh1b", [H, 512], F16)
    bias1f = din("bias1f", [2, 512], F16)
    bias1b = din("bias1b", [2, 512], F16)
    wrepf = din("wrepf", [H, 128], F16)
    wrepb = din("wrepb", [H, 128], F16)
    ident = din("ident", [128, 128], F16)
    battc = din("battc", [128, 1], F32)
    corr = din("corr", [128, BS], F32)
    xsT = din("xsT", [FS, BS], F32)
    ws0 = din("ws0", [FS, 16], F32)
    ws1 = din("ws1", [16, 16], F32)
    bs0 = din("bs0", [16, 1], F32)
    bs1 = din("bs1", [16, 1], F32)
    wc1s = din("wc1s", [16, 64], F32)
    wc1f = din("wc1f", [H, 64], F32)
    wc1b = din("wc1b", [H, 64], F32)
    bc1 = din("bc1", [64, 1], F32)
    wc2 = din("wc2", [64, 32], F32)
    bc2 = din("bc2", [32, 1], F32)

    outT = nc.dram_tensor("outT", [32, BS], F32, kind="ExternalOutput").ap()

    with tile.TileContext(nc) as tc, ExitStack() as gctx:
        gpool = gctx.enter_context(tc.tile_pool(name="glob", bufs=1))
        hzero = gpool.tile([128, 2 * BS], F16, tag="hzero")
        nc.gpsimd.memset(hzero[:], 0.0)
        for _rep in range(reps):
            with ExitStack() as rctx:
                persist = rctx.enter_context(tc.tile_pool(name="persist", bufs=1))

                # ---- persistent weights / small tensors ----
                s_w = {}
                for nm, ap_, shp, dt in (
                    ("wx0f", wx0f, [F + 2, 512], F16),
                    ("wx0b", wx0b, [F + 2, 512], F16),
                    ("wh0f", wh0f, [H, 512], F16),
                    ("wh0b", wh0b, [H, 512], F16),
                    ("xdT", xdT, [F + 2, CWX], F16),
                    ("wx1ft", wx1ft, [H, 512], F16),
                    ("wx1fb", wx1fb, [H, 512], F16),
                    ("wh1f", wh1f, [H, 512], F16),
                    ("wx1bt", wx1bt, [H, 512], F16),
                    ("wx1bb", wx1bb, [H, 512], F16),
                    ("wh1b", wh1b, [H, 512], F16),
                    ("bias1f", bias1f, [2, 512], F16),
                    ("bias1b", bias1b, [2, 512], F16),
                    ("wrepf", wrepf, [H, 128], F16),
                    ("wrepb", wrepb, [H, 128], F16),
                    ("ident", ident, [128, 128], F16),
                    ("battc", battc, [128, 1], F32),
                    ("corr", corr, [128, BS], F32),
                    ("xsT", xsT, [FS, BS], F32),
                    ("ws0", ws0, [FS, 16], F32),
                    ("ws1", ws1, [16, 16], F32),
                    ("bs0", bs0, [16, 1], F32),
                    ("bs1", bs1, [16, 1], F32),
                    ("wc1s", wc1s, [16, 64], F32),
                    ("wc1f", wc1f, [H, 64], F32),
                    ("wc1b", wc1b, [H, 64], F32),
                    ("bc1", bc1, [64, 1], F32),
                    ("wc2", wc2, [64, 32], F32),
                    ("bc2", bc2, [32, 1], F32),
                ):
                    if nm == "xdT":
                        s_w[nm] = persist.tile(shp, dt, tag="t_" + nm,
                                               name="t_" + nm)
                        # alternating head/tail chunks: fw needs the head
                        # first, bw the tail
                        NCH = 16
                        bounds = [round(i * CWX / NCH) for i in range(NCH + 1)]
                        lo, hi = 0, NCH - 1
                        ordr = []
                        while lo <= hi:
                            ordr.append(lo)
                            if hi != lo:
                                ordr.append(hi)
                            lo += 1
                            hi -= 1
                        for ci in ordr:
                            a, b_ = bounds[ci], bounds[ci + 1]
                            if b_ > a:
                                nc.sync.dma_start(out=s_w[nm][:, a:b_],
                                                  in_=ap_[:, a:b_])
                    else:
                        s_w[nm] = persist.tile(shp, dt, tag="t_" + nm,
                                               name="t_" + nm)
                        nc.sync.dma_start(out=s_w[nm][:], in_=ap_[:])

                fw0T = persist.tile([128, CWW], F16, tag="fw0T")
                bw0T = persist.tile([128, CWW], F16, tag="bw0T")
                sT = persist.tile([16, BS], F32, tag="sT")

                fw0_pitch = fw0T[:].ap[0][0]
                bw0_pitch = bw0T[:].ap[0][0]

                # ---- static branch ----
                with tc.tile_pool(name="ps_static", bufs=1, space="PSUM") as pss:
                    ps1 = pss.tile([16, BS], F32, tag="pst1")
                    nc.tensor.matmul(ps1[:], s_w["ws0"][:], s_w["xsT"][:],
                                     start=True, stop=True)
                    s0 = persist.tile([16, BS], F32, tag="s0tmp")
                    nc.scalar.activation(s0[:], ps1[:], AF.Relu, bias=s_w["bs0"][:])
                    ps2 = pss.tile([16, BS], F32, tag="pst2")
                    nc.tensor.matmul(ps2[:], s_w["ws1"][:], s0[:], start=True,
                                     stop=True)
                    nc.scalar.activation(sT[:], ps2[:], AF.Relu, bias=s_w["bs1"][:])

                def emit_stage(stage, sctx, h1all=None, att_state=None):
                    """Emit all rounds of one stage (0 or 1)."""
                    pgate = sctx.enter_context(
                        tc.tile_pool(name=f"psg{stage}", bufs=1, space="PSUM"))
                    pq = sctx.enter_context(
                        tc.tile_pool(name=f"pq{stage}", bufs=1, space="PSUM"))
                    tfi = pq.tile([128, 512], F32, tag=f"tfi{stage}",
                                  name=f"tfi{stage}")
                    t1th = pq.tile([128, 512], F32, tag=f"t1th{stage}",
                                   name=f"t1th{stage}")
                    psig = sctx.enter_context(
                        tc.tile_pool(name=f"sig{stage}", bufs=3))
                    ptmp = sctx.enter_context(
                        tc.tile_pool(name=f"tmp{stage}", bufs=4))
                    pd = sctx.enter_context(tc.tile_pool(name=f"d{stage}", bufs=1))
                    dp = [pd.tile([128, 2 * BS], F16, tag=f"d{stage}_{k}",
                                  name=f"d{stage}_{k}") for k in range(2)]
                    hpair = [pd.tile([128, 2 * BS], F16, tag=f"hp{stage}_{k}",
                                     name=f"hp{stage}_{k}") for k in range(2)]
                    for k in range(2):
                        nc.gpsimd.memset(dp[k][:], 0.0)
                        nc.gpsimd.memset(hpair[k][:], 0.0)

                    if stage == 1:
                        patt = sctx.enter_context(
                            tc.tile_pool(name="attp", bufs=3))
                        psl_pool = sctx.enter_context(
                            tc.tile_pool(name="psl", bufs=1, space="PSUM"))
                        h1_pitch = h1all[:].ap[0][0]

                    def f_addr(j):
                        # fw slab idx j within stage-1 store
                        t = steps[j]
                        if j < half:
                            return off_kf[t]
                        return SCF + (j % 3) * BS

                    def b_addr(j):
                        t = steps[j]
                        if j > half:
                            return off_kb[t]
                        return SCB + ((ns - 1 - j) % 3) * BS

                    def emit_attention(r, last=False):
                        """Attention for the slab pair completed at round r
                        (emitted one round later to keep PE free-running)."""
                        jhi, jlo = r, ns - 1 - r
                        m = int(M[steps[jhi]])
                        single = (jhi == jlo)
                        nsl = 1 if single else 2
                        hfr = hpair[r % 2]  # round r's fresh outputs
                        psl = psl_pool.tile([128, 512], F32, tag="psl")
                        # slab order [lo, hi] so both manual-AP pairs ascend;
                        # the round's fresh sides read the pair buffer (no
                        # wait on the Pool slab copies)
                        pairs = ([(jlo, 0), (jhi, 1)] if not single
                                 else [(jhi, 0)])
                        for j, sl in pairs:
                            f_src = (hfr[:, 0:m] if j == jhi
                                     else _ap2(h1all[:], f_addr(j), h1_pitch, m))
                            b_src = (hfr[:, BS:BS + m] if j == jlo or single
                                     else _ap2(h1all[:], b_addr(j), h1_pitch, m))
                            nc.tensor.matmul(
                                psl[:, sl * 128:sl * 128 + m], s_w["wrepf"][:],
                                f_src, start=True, stop=False)
                            nc.tensor.matmul(
                                psl[:, sl * 128:sl * 128 + m], s_w["wrepb"][:],
                                b_src, start=False, stop=True)
                        thl = patt.tile([128, 256], F16, tag="thl")
                        esl = patt.tile([128, 256], F16, tag="esl")
                        pin = psl[:].rearrange("p (s n) -> p s n", s=4)[:, 0:nsl, 0:m]
                        tout = thl[:].rearrange("p (s n) -> p s n", s=2)[:, 0:nsl, 0:m]
                        nc.scalar.activation(tout, pin, AF.Tanh, bias=s_w["battc"][:])
                        eout = esl[:].rearrange("p (s n) -> p s n", s=2)[:, 0:nsl, 0:m]
                        nc.scalar.activation(eout, tout, AF.Exp)
                        # ws = es * slab (both dirs)
                        wsf = patt.tile([128, 256], F16, tag="wsf")
                        wsb = patt.tile([128, 256], F16, tag="wsb")
                        if single:
                            f_ap = _ap2(h1all[:], f_addr(jhi), h1_pitch, m)
                            b_ap = _ap2(h1all[:], b_addr(jhi), h1_pitch, m)
                        else:
                            f0, f1_ = f_addr(jlo), f_addr(jhi)
                            b0, b1_ = b_addr(jlo), b_addr(jhi)
                            assert f1_ > f0 and b1_ > b0
                            f_ap = _ap3(h1all[:], f0, h1_pitch, f1_ - f0, m)
                            b_ap = _ap3(h1all[:], b0, h1_pitch, b1_ - b0, m)
                        wf = wsf[:].rearrange("p (s n) -> p s n", s=2)[:, 0:nsl, 0:m]
                        wb = wsb[:].rearrange("p (s n) -> p s n", s=2)[:, 0:nsl, 0:m]
                        ein = esl[:].rearrange("p (s n) -> p s n", s=2)[:, 0:nsl, 0:m]
                        if single:
                            f_ap = _ap3(h1all[:], f_addr(jhi), h1_pitch, 1, m,
                                        nd=1)
                            b_ap = _ap3(h1all[:], b_addr(jhi), h1_pitch, 1, m,
                                        nd=1)
                        # ws mults on Pool: off the DVE critical path
                        nc.gpsimd.tensor_tensor(wf, f_ap, ein, ALU.mult)
                        nc.gpsimd.tensor_tensor(wb, b_ap, ein, ALU.mult)
                        # defer the accumulate id-MMs by one more round so
                        # they reach the PE queue head with es/ws already
                        # computed (no head-of-line stall before the critical
                        # Wh matmuls)
                        att_state["pending"].append(
                            dict(esl=esl, wsf=wsf, wsb=wsb, m=m, nsl=nsl))

                    def emit_attention_B(last=False):
                        rec = att_state["pending"].pop(0)
                        esl, wsf, wsb = rec["esl"], rec["wsf"], rec["wsb"]
                        m, nsl = rec["m"], rec["nsl"]
                        # accumulate into persistent PSUM banks via identity
                        # MMs; regions must be uniformly fresh-or-accumulating,
                        # so split on the per-bank high-water mark when the
                        # round width grows.
                        for sl in range(nsl):
                            for kind, src in (("d", esl), ("f", wsf),
                                              ("b", wsb)):
                                acc = att_state[kind]
                                mw = att_state["mw"][kind]
                                stop = (last and sl == nsl - 1)
                                if att_state["first"][kind]:
                                    nc.tensor.matmul(
                                        acc[:, 0:m], s_w["ident"][:],
                                        src[:, sl * 128:sl * 128 + m],
                                        start=True, stop=stop)
                                    att_state["first"][kind] = False
                                elif m > mw:
                                    nc.tensor.matmul(
                                        acc[:, 0:mw], s_w["ident"][:],
                                        src[:, sl * 128:sl * 128 + mw],
                                        start=False, stop=False)
                                    nc.tensor.matmul(
                                        acc[:, mw:m], s_w["ident"][:],
                                        src[:, sl * 128 + mw:sl * 128 + m],
                                        start=False, stop=stop)
                                else:
                                    nc.tensor.matmul(
                                        acc[:, 0:m], s_w["ident"][:],
                                        src[:, sl * 128:sl * 128 + m],
                                        start=False, stop=stop)
                                att_state["mw"][kind] = max(mw, m)

                    for r in range(ns):
                        if stage == 1 and len(att_state["pending"]) >= 2:
                            emit_attention_B()
                        tf = steps[r]
                        tb = steps[ns - 1 - r]
                        m = int(M[tf])
                        ps = pgate.tile([128, 1024], F32, tag=f"ps{stage}",
                                        name=f"ps{stage}")
                        # ---- gate matmuls: x-parts first, Wh last ----
                        if stage == 0:
                            for di, (wx, xo) in enumerate(
                                    ((s_w["wx0f"], off_x[tf]),
                                     (s_w["wx0b"], off_x[tb]))):
                                for c in range(4):
                                    nc.tensor.matmul(
                                        ps[:, di * 512 + c * 128:
                                           di * 512 + c * 128 + m],
                                        wx[:, c * 128:(c + 1) * 128],
                                        s_w["xdT"][:, xo:xo + m],
                                        start=(c == 0), stop=False)
                        else:
                            for di, (wt, wb_, bt, tx) in enumerate((
                                    (s_w["wx1ft"], s_w["wx1fb"], s_w["bias1f"], tf),
                                    (s_w["wx1bt"], s_w["wx1bb"], s_w["bias1b"], tb))):
                                fsl = _ap2(fw0T[:], off_w[tx], fw0_pitch, m)
                                bsl = _ap2(bw0T[:], off_w[tx], bw0_pitch, m)
                                for c in range(4):
                                    nc.tensor.matmul(
                                        ps[:, di * 512 + c * 128:
                                           di * 512 + c * 128 + m],
                                        wt[:, c * 128:(c + 1) * 128], fsl,
                                        start=(c == 0), stop=False)
                                for c in range(4):
                                    nc.tensor.matmul(
                                        ps[:, di * 512 + c * 128:
                                           di * 512 + c * 128 + m],
                                        wb_[:, c * 128:(c + 1) * 128], bsl,
                                        start=False, stop=False)
                                for c in l1_chunks:
                                    nc.tensor.matmul(
                                        ps[:, di * 512 + c * 128:
                                           di * 512 + c * 128 + m],
                                        bt[:, c * 128:(c + 1) * 128],
                                        s_w["xdT"][0:2, off_x[tx]:off_x[tx] + m],
                                        start=False, stop=False)
                        # Wh (recurrent, on the critical path); h_prev comes
                        # from the previous round's pair buffer. o-gate chunk
                        # (3) emitted last so tau_fij only waits on chunks 0-2.
                        hpv = hzero if r == 0 else hpair[(r - 1) % 2]
                        whf = s_w["wh0f"] if stage == 0 else s_w["wh1f"]
                        whb = s_w["wh0b"] if stage == 0 else s_w["wh1b"]
                        dirs = ((0, whf, hpv[:, 0:m]),
                                (1, whb, hpv[:, BS:BS + m]))
                        for c in (0, 1, 2, 3):
                            for di, wh, hprev in dirs:
                                nc.tensor.matmul(
                                    ps[:, di * 512 + c * 128:
                                       di * 512 + c * 128 + m],
                                    wh[:, c * 128:(c + 1) * 128], hprev,
                                    start=False,
                                    stop=(c == 3))
                        # attention A-part for the previous round, after the
                        # critical Wh MMs (its th/es fill the ACT idle window
                        # before this round's tau)
                        if stage == 1 and r - 1 >= r_att:
                            emit_attention(r - 1)
                        # ---- tau, PSUM-routed: f,i chunks -> PSUM (feeds the
                        # DVE tail without the SBUF read-after-write bubble),
                        # j,o chunks -> SBUF ----
                        sig = psig.tile([128, 512], F16, tag=f"sig{stage}",
                                        name=f"sig{stage}")
                        gp = ps[:].rearrange("p (d c n) -> p d c n", d=2, c=4)
                        tfir = tfi[:].rearrange("p (d c n) -> p d c n", d=2, c=2)
                        sjor = sig[:].rearrange("p (d c n) -> p d c n", d=2, c=2)
                        nc.scalar.activation(tfir[:, :, :, 0:m],
                                             gp[:, :, 0:2, 0:m], AF.Tanh)
                        nc.scalar.activation(sjor[:, :, :, 0:m],
                                             gp[:, :, 2:4, 0:m], AF.Tanh)
                        # ---- tail (merged dirs): t1, theta in PSUM ----
                        dprev, dnew = dp[r % 2], dp[(r + 1) % 2]
                        t2 = ptmp.tile([128, 256], F16, tag="t2")
                        dpr = dprev[:].rearrange("p (d n) -> p d n", d=2)
                        dnw = dnew[:].rearrange("p (d n) -> p d n", d=2)
                        t1r = t1th[:, 0:256].rearrange("p (d n) -> p d n", d=2)
                        thr = t1th[:, 256:512].rearrange("p (d n) -> p d n", d=2)
                        t2r = t2[:].rearrange("p (d n) -> p d n", d=2)
                        nc.vector.scalar_tensor_tensor(
                            t1r[:, :, 0:m], tfir[:, :, 0, 0:m], 1.0,
                            dpr[:, :, 0:m], ALU.add, ALU.mult)
                        nc.vector.scalar_tensor_tensor(
                            t2r[:, :, 0:m], tfir[:, :, 1, 0:m], 1.0,
                            sjor[:, :, 0, 0:m], ALU.add, ALU.mult)
                        nc.vector.scalar_tensor_tensor(
                            dnw[:, :, 0:m], t1r[:, :, 0:m], 0.5,
                            t2r[:, :, 0:m], ALU.mult, ALU.add)
                        nc.scalar.activation(thr[:, :, 0:m], dnw[:, :, 0:m],
                                             AF.Tanh, scale=0.5)
                        # hhat = (tau_o + 1) * th -> pair buffer (feeds next
                        # round's Wh MMs directly)
                        hcur = hpair[r % 2]
                        hcr = hcur[:].rearrange("p (d n) -> p d n", d=2)
                        nc.vector.scalar_tensor_tensor(
                            hcr[:, :, 0:m], sjor[:, :, 1, 0:m], 1.0,
                            thr[:, :, 0:m], ALU.add, ALU.mult)
                        # off-chain slab fills on Pool (consumed rounds later)
                        if stage == 0:
                            fdst = _ap2(fw0T[:], off_w[tf], fw0_pitch, m)
                            bdst = _ap2(bw0T[:], off_w[tb], bw0_pitch, m)
                        else:
                            fdst = _ap2(h1all[:], f_addr(r), h1_pitch, m)
                            bdst = _ap2(h1all[:], b_addr(ns - 1 - r),
                                        h1_pitch, m)
                        nc.gpsimd.tensor_copy(fdst, hcur[:, 0:m])
                        nc.gpsimd.tensor_copy(bdst, hcur[:, BS:BS + m])
                    # trailing attention rounds + drain deferred id-MMs
                    if stage == 1:
                        for r in range(max(r_att, ns - 1), ns):
                            emit_attention(r)
                        while att_state["pending"]:
                            emit_attention_B(
                                last=(len(att_state["pending"]) == 1))

                # ================= stage 0 =================
                with ExitStack() as sctx:
                    emit_stage(0, sctx)

                # ================= stage 1 + attention =================
                att_f = persist.tile([H, BS], F32, tag="att_f")
                att_b = persist.tile([H, BS], F32, tag="att_b")
                den_s = persist.tile([128, BS], F32, tag="den_s")
                with ExitStack() as sctx:
                    h1all = sctx.enter_context(
                        tc.tile_pool(name="h1pool", bufs=1)
                    ).tile([128, H1W], F16, tag="h1all")
                    pacc = sctx.enter_context(
                        tc.tile_pool(name="acc", bufs=1, space="PSUM"))
                    accd = pacc.tile([128, 512], F32, tag="accd")
                    accf = pacc.tile([128, 512], F32, tag="accf")
                    accb = pacc.tile([128, 512], F32, tag="accb")
                    att_state = {"d": accd, "f": accf, "b": accb,
                                 "first": {"d": True, "f": True, "b": True},
                                 "mw": {"d": 0, "f": 0, "b": 0}, "pending": []}
                    emit_stage(1, sctx, h1all=h1all, att_state=att_state)
                    # evacuate attention accumulators
                    nc.vector.tensor_tensor(den_s[:], accd[:, 0:BS],
                                            s_w["corr"][:], ALU.add)
                    rd = persist.tile([128, BS], F32, tag="rd")
                    nc.vector.reciprocal(rd[:], den_s[:])
                    nc.vector.tensor_tensor(att_f[:], accf[:, 0:BS],
                                            rd[:], ALU.mult)
                    nc.vector.tensor_tensor(att_b[:], accb[:, 0:BS],
                                            rd[:], ALU.mult)

                # ================= classifier =================
                with ExitStack() as cctx:
                    pcl = cctx.enter_context(tc.tile_pool(name="cls", bufs=1))
                    psc = cctx.enter_context(
                        tc.tile_pool(name="psum_cls", bufs=1, space="PSUM"))
                    ph = psc.tile([64, BS], F32, tag="ph")
                    nc.tensor.matmul(ph[:], s_w["wc1s"][:], sT[:], start=True,
                                     stop=False)
                    nc.tensor.matmul(ph[:], s_w["wc1f"][:], att_f[:], start=False,
                                     stop=False)
                    nc.tensor.matmul(ph[:], s_w["wc1b"][:], att_b[:], start=False,
                                     stop=True)
                    h1t = pcl.tile([64, BS], F32, tag="h1t")
                    nc.scalar.activation(h1t[:], ph[:], AF.Relu, bias=s_w["bc1"][:])
                    po = psc.tile([32, BS], F32, tag="po")
                    nc.tensor.matmul(po[:], s_w["wc2"][:], h1t[:], start=True,
                                     stop=True)
                    oT = pcl.tile([32, BS], F32, tag="oT")
                    nc.scalar.activation(oT[:], po[:], AF.Relu, bias=s_w["bc2"][:])
                    nc.sync.dma_start(out=outT[:], in_=oT[:])
                    if serialize and _rep < reps - 1:
                        # write zeros that depend on this rep's output into
                        # hzero, so the next rep's round 0 waits for it
                        nc.vector.scalar_tensor_tensor(
                            hzero[0:32, 0:BS], oT[:], 0.0, oT[:],
                            ALU.mult, ALU.mult)

    return nc


def kernel(x_static, x_dynamic, seq_len, w_s0, b_s0, w_s1, b_s1,
           Wx_f0, Wh_f0, bb_f0, Wx_b0, Wh_b0, bb_b0,
           Wx_f1, Wh_f1, bb_f1, Wx_b1, Wh_b1, bb_b1,
           w_att, b_att, w_c1, b_c1, w_c2, b_c2):
    inputs = dict(
        x_static=x_static, x_dynamic=x_dynamic, seq_len=seq_len,
        w_s0=w_s0, b_s0=b_s0, w_s1=w_s1, b_s1=b_s1,
        Wx_f0=Wx_f0, Wh_f0=Wh_f0, bb_f0=bb_f0,
        Wx_b0=Wx_b0, Wh_b0=Wh_b0, bb_b0=bb_b0,
        Wx_f1=Wx_f1, Wh_f1=Wh_f1, bb_f1=bb_f1,
        Wx_b1=Wx_b1, Wh_b1=Wh_b1, bb_b1=bb_b1,
        w_att=w_att, b_att=b_att, w_c1=w_c1, b_c1=b_c1,
        w_c2=w_c2, b_c2=b_c2,
    )
    sched, in_maps = prepare(inputs)
    nc = build_program(sched)
    split_multi_waits(nc, max_waits=1)

    trace = os.environ.get("TRN_KERNEL_TRACE", "0") == "1"
    try:
        res = run_bass_kernel_spmd(nc, in_maps, list(range(NCORES)), trace=trace)
    except ModuleNotFoundError:
        res = run_bass_kernel_spmd(nc, in_maps, list(range(NCORES)))
    if trace:
        kernel.last_results = res

    out = np.zeros((B, 32), dtype=np.float32)
    for c in range(NCORES):
        out[sched["perms"][c]] = res.results[c]["outT"].T
    return out
